# revision 8
# baseline (speedup 1.0000x reference)
"""Trainium2 Bass kernel for nn_Attention_68685116997866.

Math (per batch b; C=128, N=32768):
    A = q_w @ y + q_b,  K = k_w @ x + k_b          (pointwise convs)
    energy = [A;K] @ [A;K]^T / sqrt(2C)            ([256,256] Gram)
    e1 = relu(energy @ t1_w^T + t1_b)
    e2 = relu(e1 @ t2_w^T + t2_b)
    attn = softmax(e2, axis=-1)                    ([256,128])
    out  = (attn_top^T @ v2_w) @ y + (attn_bot^T @ v1_w) @ x
         + (attn_top^T @ v2_b + attn_bot^T @ v1_b) 1^T

Strategy: pure data-parallel over B across the 8 cores (1 batch/core).
Inputs are downcast to bf16 on host (halves HBM traffic; verified
~1e-3 rel err) and kept SBUF-resident so HBM is touched once.
Phase 1 accumulates an augmented Gram G = [A;K;1]^T-gram in PSUM over
256 column-chunks: per chunk, transposed A^T/K^T tiles are produced by
the PE (lhsT = data chunk, rhs = pre-transposed q/k weights), copied to
fp32 T-tiles, and two fp32r matmuls accumulate G (fp32r = full-rate
fp32 PE mode at free-dim >= 256). Biases enter via a rank-2 PSUM update
built from the Gram's ones-column rowsums. The tiny MLP+softmax runs
on-chip in a transposed layout so every bias lands on partitions; attn
is folded into the v-weights, and phase 2 streams
out = WyT^T @ y + WxT^T @ x + bout over 512-col chunks.
"""

import sys

for _p in ("/opt/trn_rl_repo",):
    if _p not in sys.path:
        sys.path.insert(0, _p)

import numpy as np
import ml_dtypes

import concourse.bass as bass  # noqa: F401  (AP types)
import concourse.mybir as mybir
import concourse.tile as tile
from concourse import bacc
from concourse.bass_utils import run_bass_kernel_spmd

B, C, N = 8, 128, 32768
F32 = mybir.dt.float32
F32R = mybir.dt.float32r
BF16 = mybir.dt.bfloat16
AF = mybir.ActivationFunctionType


def build_program(n=N, segs=8, gram_f32r=True):
    """Build the per-core Bass program (one batch per core)."""
    nc = bacc.Bacc(None, target_bir_lowering=False)
    seg_cols = n // segs
    n_chunks = n // 128
    chunks_per_seg = seg_cols // 128
    oc = min(512, seg_cols)
    out_chunks = n // oc

    # ---- DRAM I/O ----
    xb_d = nc.dram_tensor("xb", [128, n], BF16, kind="ExternalInput")
    yb_d = nc.dram_tensor("yb", [128, n], BF16, kind="ExternalInput")
    qt_d = nc.dram_tensor("qt", [128, 128], BF16, kind="ExternalInput")
    kt_d = nc.dram_tensor("kt", [128, 128], BF16, kind="ExternalInput")
    v1w_d = nc.dram_tensor("v1w", [128, 128], F32, kind="ExternalInput")
    v2w_d = nc.dram_tensor("v2w", [128, 128], F32, kind="ExternalInput")
    v1b_d = nc.dram_tensor("v1b", [128, 1], F32, kind="ExternalInput")
    v2b_d = nc.dram_tensor("v2b", [128, 1], F32, kind="ExternalInput")
    t1wt_d = nc.dram_tensor("t1wt", [256, 256], F32, kind="ExternalInput")
    t1b_d = nc.dram_tensor("t1b", [256, 1], F32, kind="ExternalInput")
    t2wt_d = nc.dram_tensor("t2wt", [256, 128], F32, kind="ExternalInput")
    t2b_row_d = nc.dram_tensor("t2b_row", [1, 128], F32, kind="ExternalInput")
    ones_row_d = nc.dram_tensor("ones_row", [1, 128], F32, kind="ExternalInput")
    c_row_d = nc.dram_tensor("c_row", [1, 256], F32, kind="ExternalInput")
    cn_row_d = nc.dram_tensor("cn_row", [1, 256], F32, kind="ExternalInput")
    qb_row_d = nc.dram_tensor("qb_row", [1, 128], F32, kind="ExternalInput")
    kb_row_d = nc.dram_tensor("kb_row", [1, 128], F32, kind="ExternalInput")
    ident_d = nc.dram_tensor("ident", [128, 128], F32, kind="ExternalInput")
    onespad_d = nc.dram_tensor("onespad", [128, 2], F32R if True else F32, kind="ExternalInput")
    out_d = nc.dram_tensor("out", [128, n], F32, kind="ExternalOutput")

    tdt = F32R if gram_f32r else F32

    with tile.TileContext(nc) as tc:
        with (
            tc.tile_pool(name="const", bufs=1) as constp,
            tc.tile_pool(name="data", bufs=1) as datap,
            tc.tile_pool(name="tbuf", bufs=1) as tbufp,
            tc.tile_pool(name="work", bufs=1) as workp,
            tc.tile_pool(name="ostage", bufs=3) as ostagep,
            tc.tile_pool(name="gacc", bufs=1, space="PSUM") as gaccp,
            tc.tile_pool(name="pp", bufs=2, space="PSUM") as ppp,
            tc.tile_pool(name="ops", bufs=2, space="PSUM") as opsp,
        ):
            # ---- constants to SBUF ----
            qt_sb = constp.tile([128, 128], BF16, tag="qt")
            nc.sync.dma_start(qt_sb, qt_d[:, :])
            kt_sb = constp.tile([128, 128], BF16, tag="kt")
            nc.sync.dma_start(kt_sb, kt_d[:, :])
            v1w_sb = constp.tile([128, 128], F32, tag="v1w")
            nc.sync.dma_start(v1w_sb, v1w_d[:, :])
            v2w_sb = constp.tile([128, 128], F32, tag="v2w")
            nc.sync.dma_start(v2w_sb, v2w_d[:, :])
            v1b_sb = constp.tile([128, 1], F32, tag="v1b")
            nc.sync.dma_start(v1b_sb, v1b_d[:, :])
            v2b_sb = constp.tile([128, 1], F32, tag="v2b")
            nc.sync.dma_start(v2b_sb, v2b_d[:, :])
            t1wt_sb = constp.tile([128, 2, 256], F32, tag="t1wt")
            nc.sync.dma_start(
                t1wt_sb, t1wt_d.ap().rearrange("(ko ki) j -> ki ko j", ki=128)
            )
            t1b_sb = constp.tile([128, 2], F32, tag="t1b")
            nc.sync.dma_start(
                t1b_sb, t1b_d.ap().rearrange("(ko ki) one -> ki (ko one)", ki=128)
            )
            t2wt_sb = constp.tile([128, 2, 128], F32, tag="t2wt")
            nc.sync.dma_start(
                t2wt_sb, t2wt_d.ap().rearrange("(ko ki) j -> ki ko j", ki=128)
            )
            t2b_row_sb = constp.tile([1, 128], F32, tag="t2br")
            nc.sync.dma_start(t2b_row_sb, t2b_row_d[:, :])
            ones_row_sb = constp.tile([1, 128], F32, tag="onesr")
            nc.sync.dma_start(ones_row_sb, ones_row_d[:, :])
            c_row_sb = constp.tile([1, 256], F32, tag="crow")
            nc.sync.dma_start(c_row_sb, c_row_d[:, :])
            cn_row_sb = constp.tile([1, 256], F32, tag="cnrow")
            nc.sync.dma_start(cn_row_sb, cn_row_d[:, :])
            qb_row_sb = constp.tile([1, 128], F32, tag="qbr")
            nc.sync.dma_start(qb_row_sb, qb_row_d[:, :])
            kb_row_sb = constp.tile([1, 128], F32, tag="kbr")
            nc.sync.dma_start(kb_row_sb, kb_row_d[:, :])
            ident_sb = constp.tile([128, 128], F32, tag="ident")
            nc.sync.dma_start(ident_sb, ident_d[:, :])

            # ---- resident input segments (bf16) ----
            ysegs = [
                datap.tile([128, seg_cols], BF16, tag=f"yseg{s}", name=f"yseg{s}")
                for s in range(segs)
            ]
            xsegs = [
                datap.tile([128, seg_cols], BF16, tag=f"xseg{s}", name=f"xseg{s}")
                for s in range(segs)
            ]
            for s in range(segs):
                nc.sync.dma_start(ysegs[s], yb_d[:, s * seg_cols : (s + 1) * seg_cols])
                nc.sync.dma_start(xsegs[s], xb_d[:, s * seg_cols : (s + 1) * seg_cols])

            # ---- T tiles (double-buffered by parity), ones col persistent ----
            T0 = tbufp.tile([128, 258], tdt, tag="T0")
            T1 = tbufp.tile([128, 258], tdt, tag="T1")
            nc.sync.dma_start(T0[:, 256:258], onespad_d[:, :].bitcast(tdt))
            nc.sync.dma_start(T1[:, 256:258], onespad_d[:, :].bitcast(tdt))

            G_top = gaccp.tile([128, 258], F32, tag="gtop")
            G_bot = gaccp.tile([128, 258], F32, tag="gbot")

            # ---- phase 1: Gram accumulation ----
            for g in range(n_chunks):
                s, i = divmod(g, chunks_per_seg)
                yc = ysegs[s][:, i * 128 : (i + 1) * 128]
                xc = xsegs[s][:, i * 128 : (i + 1) * 128]
                at = ppp.tile([128, 256], F32, tag="at")
                bt = ppp.tile([128, 256], F32, tag="bt")
                nc.tensor.matmul(at[:, 0:128], yc, qt_sb, start=True, stop=True)
                nc.tensor.matmul(bt[:, 0:128], xc, kt_sb, start=True, stop=True)
                T = T0 if g % 2 == 0 else T1
                nc.vector.tensor_copy(T[:, 0:128], at[:, 0:128])
                nc.vector.tensor_copy(T[:, 128:256], bt[:, 0:128])
                nc.tensor.matmul(
                    G_top, T[:, 0:128], T[:, :],
                    start=(g == 0), stop=False, skip_group_check=True,
                )
                nc.tensor.matmul(
                    G_bot, T[:, 128:256], T[:, :],
                    start=(g == 0), stop=False, skip_group_check=True,
                )

            # ---- postlude: bias rank-2 correction ----
            # G rows hold pure-matmul gram; col 256 = rowsums (A1, B1).
            # energy = G0 + m1 c^T + c (m1 + n c)^T, c = [q_b;k_b].
            a1_col = workp.tile([128, 1], F32, tag="a1c")
            nc.vector.tensor_copy(a1_col, G_top[:, 256:257])
            b1_col = workp.tile([128, 1], F32, tag="b1c")
            nc.vector.tensor_copy(b1_col, G_bot[:, 256:257])
            rowa = ppp.tile([1, 128], F32, tag="at")
            nc.tensor.matmul(rowa, a1_col, ident_sb, start=True, stop=True)
            rowb = ppp.tile([1, 128], F32, tag="bt")
            nc.tensor.matmul(rowb, b1_col, ident_sb, start=True, stop=True)
            m1_row = workp.tile([1, 256], F32, tag="m1r")
            nc.vector.tensor_copy(m1_row[:, 0:128], rowa)
            nc.vector.tensor_copy(m1_row[:, 128:256], rowb)

            u_row = workp.tile([1, 256], F32, tag="urow")
            nc.vector.tensor_add(u_row, m1_row, cn_row_sb)
            nc.tensor.matmul(
                G_top[:, 0:256], m1_row[:, 0:128], c_row_sb,
                start=False, stop=False, skip_group_check=True,
            )
            nc.tensor.matmul(
                G_top[:, 0:256], qb_row_sb, u_row,
                start=False, stop=True, skip_group_check=True,
            )
            nc.tensor.matmul(
                G_bot[:, 0:256], m1_row[:, 128:256], c_row_sb,
                start=False, stop=False, skip_group_check=True,
            )
            nc.tensor.matmul(
                G_bot[:, 0:256], kb_row_sb, u_row,
                start=False, stop=True, skip_group_check=True,
            )

            # energy to SBUF (1/sqrt(2C) folded into t1wt on host)
            E_top = workp.tile([128, 256], F32, tag="etop")
            nc.vector.tensor_copy(E_top, G_top[:, 0:256])
            E_bot = workp.tile([128, 256], F32, tag="ebot")
            nc.vector.tensor_copy(E_bot, G_bot[:, 0:256])

            # ---- MLP layer 1 (transposed): e1T = relu(t1wt^T @ E + t1b) ----
            e1T_sb = []
            for r in range(2):
                ps = ppp.tile([128, 256], F32, tag="at" if r == 0 else "bt")
                nc.tensor.matmul(
                    ps, t1wt_sb[:, 0, r * 128 : (r + 1) * 128], E_top,
                    start=True, stop=False,
                )
                nc.tensor.matmul(
                    ps, t1wt_sb[:, 1, r * 128 : (r + 1) * 128], E_bot,
                    start=False, stop=True,
                )
                sb = workp.tile([128, 256], F32, tag=f"e1t{r}")
                nc.scalar.activation(sb, ps, AF.Relu, bias=t1b_sb[:, r : r + 1])
                e1T_sb.append(sb)

            # ---- MLP layer 2 + softmax: attn rows r*128..r*128+127 ----
            attn = []
            for r in range(2):
                ps = ppp.tile([128, 128], F32, tag="at" if r == 0 else "bt")
                nc.tensor.matmul(
                    ps, e1T_sb[0][:, r * 128 : (r + 1) * 128], t2wt_sb[:, 0, :],
                    start=True, stop=False,
                )
                nc.tensor.matmul(
                    ps, e1T_sb[1][:, r * 128 : (r + 1) * 128], t2wt_sb[:, 1, :],
                    start=False, stop=False,
                )
                nc.tensor.matmul(
                    ps, ones_row_sb, t2b_row_sb,
                    start=False, stop=True, skip_group_check=True,
                )
                e2 = workp.tile([128, 128], F32, tag=f"e2_{r}")
                nc.scalar.activation(e2, ps, AF.Relu)
                mneg = workp.tile([128, 1], F32, tag=f"mx{r}")
                nc.vector.tensor_reduce(
                    mneg, e2, axis=mybir.AxisListType.X,
                    op=mybir.AluOpType.max, negate=True,
                )
                p_t = workp.tile([128, 128], F32, tag=f"pt{r}")
                ssum = workp.tile([128, 1], F32, tag=f"sm{r}")
                nc.scalar.activation(p_t, e2, AF.Exp, bias=mneg, accum_out=ssum)
                rcp = workp.tile([128, 1], F32, tag=f"rc{r}")
                nc.vector.reciprocal(rcp, ssum)
                a_t = workp.tile([128, 128], F32, tag=f"attn{r}")
                nc.vector.tensor_scalar_mul(a_t, p_t, rcp)
                attn.append(a_t)

            # ---- fold attn into v-weights ----
            wy_ps = ppp.tile([128, 128], F32, tag="at")
            nc.tensor.matmul(wy_ps, v2w_sb, attn[0], start=True, stop=True)
            wyt_sb = workp.tile([128, 128], BF16, tag="wyt")
            nc.vector.tensor_copy(wyt_sb, wy_ps)
            wx_ps = ppp.tile([128, 128], F32, tag="bt")
            nc.tensor.matmul(wx_ps, v1w_sb, attn[1], start=True, stop=True)
            wxt_sb = workp.tile([128, 128], BF16, tag="wxt")
            nc.vector.tensor_copy(wxt_sb, wx_ps)
            bout_ps = ppp.tile([128, 1], F32, tag="at")
            nc.tensor.matmul(bout_ps, attn[0], v2b_sb, start=True, stop=False)
            nc.tensor.matmul(bout_ps, attn[1], v1b_sb, start=False, stop=True)
            bout_sb = workp.tile([128, 1], F32, tag="bout")
            nc.vector.tensor_copy(bout_sb, bout_ps)

            # ---- phase 2: out = WyT^T @ y + WxT^T @ x + bout ----
            for j in range(out_chunks):
                s, off = divmod(j * oc, seg_cols)
                ps = opsp.tile([128, 512], F32, tag="ops")
                nc.tensor.matmul(
                    ps[:, 0:oc], wyt_sb, ysegs[s][:, off : off + oc],
                    start=True, stop=False,
                )
                nc.tensor.matmul(
                    ps[:, 0:oc], wxt_sb, xsegs[s][:, off : off + oc],
                    start=False, stop=True,
                )
                ot = ostagep.tile([128, 512], F32, tag="ot")
                nc.scalar.activation(ot[:, 0:oc], ps[:, 0:oc], AF.Identity, bias=bout_sb)
                nc.sync.dma_start(out_d[:, j * oc : (j + 1) * oc], ot[:, 0:oc])

    nc.finalize()
    return nc


_PROGRAM_CACHE = {}


def get_program(n=N, segs=8, gram_f32r=True):
    key = (n, segs, gram_f32r)
    if key not in _PROGRAM_CACHE:
        _PROGRAM_CACHE[key] = build_program(n, segs, gram_f32r)
    return _PROGRAM_CACHE[key]


def prep_in_maps(inputs, n=N):
    """Host-side prep: shard over batch, pre-transpose/fold weights."""
    bf = ml_dtypes.bfloat16
    f32 = np.float32
    x, y = np.asarray(inputs["x"]), np.asarray(inputs["y"])
    qw, qb = np.asarray(inputs["q_w"]), np.asarray(inputs["q_b"])
    kw, kb = np.asarray(inputs["k_w"]), np.asarray(inputs["k_b"])
    v1w, v1b = np.asarray(inputs["v1_w"]), np.asarray(inputs["v1_b"])
    v2w, v2b = np.asarray(inputs["v2_w"]), np.asarray(inputs["v2_b"])
    t1w, t1b = np.asarray(inputs["t1_w"]), np.asarray(inputs["t1_b"])
    t2w, t2b = np.asarray(inputs["t2_w"]), np.asarray(inputs["t2_b"])

    s = np.sqrt(f32(2 * C))
    cvec = np.concatenate([qb, kb]).astype(f32)
    shared = {
        "qt": np.ascontiguousarray(qw.T).astype(bf),
        "kt": np.ascontiguousarray(kw.T).astype(bf),
        "v1w": np.ascontiguousarray(v1w).astype(f32),
        "v2w": np.ascontiguousarray(v2w).astype(f32),
        "v1b": v1b.reshape(128, 1).astype(f32),
        "v2b": v2b.reshape(128, 1).astype(f32),
        "t1wt": np.ascontiguousarray(t1w.T / s).astype(f32),
        "t1b": t1b.reshape(256, 1).astype(f32),
        "t2wt": np.ascontiguousarray(t2w.T).astype(f32),
        "t2b_row": t2b.reshape(1, 128).astype(f32),
        "ones_row": np.ones((1, 128), f32),
        "c_row": cvec.reshape(1, 256),
        "cn_row": (f32(n) * cvec).reshape(1, 256),
        "qb_row": qb.reshape(1, 128).astype(f32),
        "kb_row": kb.reshape(1, 128).astype(f32),
        "ident": np.eye(128, dtype=f32),
        "onespad": np.concatenate(
            [np.ones((128, 1), f32), np.zeros((128, 1), f32)], axis=1
        ),
    }
    in_maps = []
    for b in range(B):
        m = dict(shared)
        m["xb"] = np.ascontiguousarray(x[b, :, :n]).astype(bf)
        m["yb"] = np.ascontiguousarray(y[b, :, :n]).astype(bf)
        in_maps.append(m)
    return in_maps


def kernel(**inputs) -> np.ndarray:
    nc = get_program()
    in_maps = prep_in_maps(inputs)
    res = run_bass_kernel_spmd(nc, in_maps, core_ids=list(range(B)))
    return np.stack([res.results[b]["out"] for b in range(B)]).astype(np.float32)
